# revision 15
# baseline (speedup 1.0000x reference)
"""Multi-head attention (B=4, S=2048, D=1024, H=16) on 8 trn2 NeuronCores.

Sharding: (batch, head-group) -> 8 shards of (1 batch x 8 heads). Zero
cross-core communication: each core computes Q/K/V projections for its 8
heads, full attention over S=2048, and a partial output projection
(row-split Wo); the host sums the two head-group partials per batch.

v2: head-PAIR packed attention. The PE array is only half-used by a
single head's matmuls (scores contract over dh=64 -> 64 rows; PV emits
65 output cols). Heads 2t/2t+1 are processed together:
  scores: row-tiled pair  -- head A weights in PE rows 0-63, head B in
          rows 64-127 (tile_position auto from base partition); the two
          matmuls stream concurrently.
  PV:     col-tiled pair  -- V_A in PE cols 0-63 -> ctx partitions 0-63,
          V_B in cols 64-127 -> partitions 64-127; one [128,1024] PSUM
          tile holds both heads' ctx.
The softmax denominator (previously a free 65th "ones" column on V, for
which the packed array has no room) is instead computed by a DVE running
sum of the pexp tiles (fp16, 2 elem/cyc) folded by a tiny [128,1]-ones
matmul, then reciprocal + broadcast as before.

Matmul operands are fp16 (fp32 PSUM accumulation; fp16's 10 mantissa
bits survive the PE's internal FP22). fp8 was considered and rejected:
random-walk signals keep the full per-element quantization noise
(~4-6% rms), far over the 2e-2 gate.
"""

import numpy as np

import concourse.bass as bass
import concourse.tile as tile
from concourse import bacc, mybir
from concourse.bass_utils import run_bass_kernel_spmd

F32 = mybir.dt.float32
F16 = mybir.dt.float16
AF = mybir.ActivationFunctionType

B, S, D = 4, 2048, 1024
HPC = 8          # heads per core
DHT = 512        # head dims per core (8 * 64)
NDT = D // 128   # 8 d-tiles (contraction tiles for projections)
NHT = DHT // 128  # 4 head-pairs per core
NST = S // 128   # 16 s-tiles
NSB = S // 512   # 4 s-blocks
N_CORES = 8


def build_nc():
    nc = bacc.Bacc(None, target_bir_lowering=False)

    xq = nc.declare_dram_parameter("xq_t", [D, S], F16, isOutput=False)
    xk = nc.declare_dram_parameter("xk_t", [D, S], F16, isOutput=False)
    xv = nc.declare_dram_parameter("xv_t", [D, S], F16, isOutput=False)
    wq = nc.declare_dram_parameter("wq", [D, DHT], F16, isOutput=False)
    wk = nc.declare_dram_parameter("wk", [D, DHT], F16, isOutput=False)
    wv = nc.declare_dram_parameter("wv", [D, DHT], F16, isOutput=False)
    wo = nc.declare_dram_parameter("wo", [DHT, D], F16, isOutput=False)
    bq = nc.declare_dram_parameter("bq", [DHT], F32, isOutput=False)
    bk = nc.declare_dram_parameter("bk", [DHT], F32, isOutput=False)
    bv = nc.declare_dram_parameter("bv", [DHT], F32, isOutput=False)
    ot = nc.declare_dram_parameter("o_t", [D, S], F32, isOutput=True)

    # DRAM views tiled to 128 partitions
    xq_v = xq.rearrange("(t p) s -> p t s", p=128)
    xk_v = xk.rearrange("(t p) s -> p t s", p=128)
    xv_v = xv.rearrange("(t p) s -> p t s", p=128)
    wq_v = wq.rearrange("(t p) n -> p t n", p=128)
    wk_v = wk.rearrange("(t p) n -> p t n", p=128)
    wv_v = wv.rearrange("(t p) n -> p t n", p=128)
    wo_v = wo.rearrange("(t p) n -> p t n", p=128)
    ot_v = ot.rearrange("(t p) s -> t p s", p=128)

    with tile.TileContext(nc) as tc:
        with (
            tc.tile_pool(name="persist", bufs=1) as persist,
            tc.tile_pool(name="outp", bufs=4) as outp,
            tc.tile_pool(name="w3", bufs=1) as w3,
            tc.tile_pool(name="xs", bufs=4) as xs,
            tc.tile_pool(name="pexp_p", bufs=8) as pexp_p,
            tc.tile_pool(name="accp", bufs=4) as accp,
            tc.tile_pool(name="norm", bufs=1) as norm,
            tc.tile_pool(name="small", bufs=2) as small,
            tc.tile_pool(name="ps_st", bufs=2, space="PSUM") as ps_st,
            tc.tile_pool(name="ps_ctx", bufs=1, space="PSUM") as ps_ctx,
            tc.tile_pool(name="ps_o", bufs=2, space="PSUM") as ps_o,
        ):
            KT = persist.tile([128, NHT, S], F16)        # K^T  [dh-pair, s]
            QT = persist.tile([128, NHT, S], F16)        # Q^T  [dh-pair, s]
            Vt = persist.tile([128, NST, NHT, 128], F16)  # V natural, pair-packed
            ctxn = persist.tile([128, NHT, S], F16)      # normalized ctx^T
            wo_sb = persist.tile([128, NHT, D], F16)
            bq_sb = persist.tile([128, NHT], F32)
            bk_sb = persist.tile([128, NHT], F32)
            bv_bc = persist.tile([128, NHT, 128], F32)
            ones = persist.tile([128, 1], F16)           # den fold column
            denrow = persist.tile([1, 2048], F32)
            rinvrow = persist.tile([1, 2048], F32)
            wq_sb = w3.tile([128, NDT, DHT], F16)
            wk_sb = w3.tile([128, NDT, DHT], F16)
            wv_sb = w3.tile([128, NDT, DHT], F16)

            nc.sync.dma_start(out=bq_sb, in_=bq.rearrange("(t p) -> p t", p=128))
            nc.sync.dma_start(out=bk_sb, in_=bk.rearrange("(t p) -> p t", p=128))
            nc.sync.dma_start(
                out=bv_bc,
                in_=bv.rearrange("(hp c) -> hp c", c=128).partition_broadcast(128),
            )
            nc.vector.memset(ones, 1.0)
            for dt in range(NDT):  # split DMAs across queues; K first
                nc.sync.dma_start(out=wk_sb[:, dt, :], in_=wk_v[:, dt, :])

            def emit_proj(kind, sb):
                ssl = slice(sb * 512, (sb + 1) * 512)
                xv_ap = {"k": xk_v, "v": xv_v, "q": xq_v}[kind]
                xst = xs.tile([128, NDT, 512], F16, tag="xs")
                for dt in range(NDT):
                    nc.sync.dma_start(out=xst[:, dt, :], in_=xv_ap[:, dt, ssl])
                if kind == "v":
                    # V projection, natural layout: the X^T tile is
                    # stationary so out[s-tile, dh] has s on partitions
                    for su in range(4):
                        pso = ps_o.tile([128, 512], F32, tag="po")
                        for dt in range(NDT):
                            nc.tensor.matmul(
                                pso[:, :],
                                xst[:, dt, bass.ts(su, 128)],
                                wv_sb[:, dt, :],
                                start=(dt == 0),
                                stop=(dt == NDT - 1),
                            )
                        nc.vector.tensor_add(
                            out=Vt[:, sb * 4 + su, :, :],
                            in0=pso.rearrange("p (hp c) -> p hp c", c=128),
                            in1=bv_bc,
                        )
                else:
                    w_sb = wk_sb if kind == "k" else wq_sb
                    dst = KT if kind == "k" else QT
                    b_sb = bk_sb if kind == "k" else bq_sb
                    # K^T / Q^T: out[dh-tile, s-blk] = W^T-contract X^T
                    for ht in range(NHT):
                        ps = ps_o.tile([128, 512], F32, tag="po")
                        for dt in range(NDT):
                            nc.tensor.matmul(
                                ps[:, :],
                                w_sb[:, dt, bass.ts(ht, 128)],
                                xst[:, dt, :],
                                start=(dt == 0),
                                stop=(dt == NDT - 1),
                            )
                        # DVE (not ACT) so the scalar engine stays
                        # free for the softmax exps
                        nc.vector.tensor_scalar_add(
                            out=dst[:, ht, ssl],
                            in0=ps[:, :],
                            scalar1=b_sb[:, ht : ht + 1],
                        )

            def emit_attention(qp):
                q0 = qp * 1024
                qsl = slice(q0, q0 + 1024)
                for hp in range(NHT):
                    ctx = ps_ctx.tile([128, 1024], F32, tag="ctx")
                    accA = accp.tile([128, 1024], F16, tag="acc")
                    accB = accp.tile([128, 1024], F16, tag="acc")
                    for sk in range(NST):
                        stA = ps_st.tile([128, 1024], F32, tag="st")
                        stB = ps_st.tile([128, 1024], F32, tag="st")
                        ksl = bass.ts(sk, 128)
                        # Row-tiled pairs (head A rows 0-63, B rows 64-127).
                        # j0/j1 form one accumulation group per st tile so
                        # only the final MM carries a completion signal, and
                        # B (whose buffer-recycle wait is the strictest) goes
                        # first so A's waits dedupe away -- both required for
                        # the pair to stream concurrently in the PE array.
                        for j in range(2):
                            jq = slice(q0 + j * 512, q0 + (j + 1) * 512)
                            nc.tensor.matmul(
                                stB[:, bass.ts(j, 512)],
                                KT[64:128, hp, ksl],
                                QT[64:128, hp, jq],
                                start=True,
                                stop=(j == 1),
                                skip_group_check=True,
                            )
                            nc.tensor.matmul(
                                stA[:, bass.ts(j, 512)],
                                KT[0:64, hp, ksl],
                                QT[0:64, hp, jq],
                                start=True,
                                stop=(j == 1),
                                skip_group_check=True,
                            )
                        pexpA = pexp_p.tile([128, 1024], F16, tag="pexp")
                        pexpB = pexp_p.tile([128, 1024], F16, tag="pexp")
                        nc.scalar.activation(
                            out=pexpA, in_=stA, func=AF.Exp, scale=0.125
                        )
                        nc.scalar.activation(
                            out=pexpB, in_=stB, func=AF.Exp, scale=0.125
                        )
                        # softmax-denominator partial sums (free-dim cheap
                        # on DVE fp16; folded across partitions at the end)
                        # den partial sums: A chain on DVE, B chain on the
                        # otherwise-idle gpsimd engine
                        if sk == 0:
                            nc.vector.tensor_copy(out=accA, in_=pexpA)
                            nc.gpsimd.tensor_copy(out=accB, in_=pexpB)
                        else:
                            nc.vector.tensor_add(out=accA, in0=accA, in1=pexpA)
                            nc.gpsimd.tensor_add(out=accB, in0=accB, in1=pexpB)
                        for j in range(2):
                            # col-tiled pair: ctx[0:64]=head A, [64:128]=head B
                            nc.tensor.matmul(
                                ctx[0:64, bass.ts(j, 512)],
                                Vt[:, sk, hp, 0:64],
                                pexpA[:, bass.ts(j, 512)],
                                start=(sk == 0),
                                stop=(sk == NST - 1),
                                skip_group_check=True,
                            )
                            nc.tensor.matmul(
                                ctx[64:128, bass.ts(j, 512)],
                                Vt[:, sk, hp, 64:128],
                                pexpB[:, bass.ts(j, 512)],
                                start=(sk == 0),
                                stop=(sk == NST - 1),
                                skip_group_check=True,
                            )
                    # drain ctx PSUM quickly to free the bank for the next
                    # pair; fold acc across partitions into den rows with a
                    # [128,1]-ones matmul; reciprocal runs 128-wide after a
                    # DMA reshape, then per-head halves broadcast back.
                    ctxcA = norm.tile([64, 1024], F32, tag="ctxcA")
                    ctxcB = norm.tile([64, 1024], F32, tag="ctxcB")
                    nc.vector.tensor_copy(out=ctxcA, in_=ctx[0:64, :])
                    nc.vector.tensor_copy(out=ctxcB, in_=ctx[64:128, :])
                    for hd, acc in ((0, accA), (1, accB)):
                        for j in range(2):
                            psd = ps_o.tile([1, 512], F32, tag="po")
                            nc.tensor.matmul(
                                psd[:, :],
                                ones,
                                acc[:, bass.ts(j, 512)],
                                start=True,
                                stop=True,
                            )
                            nc.vector.tensor_copy(
                                out=denrow[:, hd * 1024 + j * 512 : hd * 1024 + (j + 1) * 512],
                                in_=psd,
                            )
                    rr = small.tile([128, 16], F32, tag="rr")
                    nc.sync.dma_start(out=rr, in_=denrow)
                    rrv = small.tile([128, 16], F32, tag="rrv")
                    nc.vector.reciprocal(out=rrv, in_=rr)
                    nc.sync.dma_start(out=rinvrow, in_=rrv)
                    rbcA = norm.tile([64, 1024], F32, tag="rbcA")
                    rbcB = norm.tile([64, 1024], F32, tag="rbcB")
                    nc.gpsimd.partition_broadcast(rbcA, rinvrow[:, 0:1024])
                    nc.gpsimd.partition_broadcast(rbcB, rinvrow[:, 1024:2048])
                    nc.vector.tensor_mul(
                        out=ctxn[0:64, hp, qsl], in0=ctxcA, in1=rbcA
                    )
                    nc.vector.tensor_mul(
                        out=ctxn[64:128, hp, qsl], in0=ctxcB, in1=rbcB
                    )
                # output projection for this q-block
                for dot in range(8):
                    for j in range(2):
                        pso = ps_o.tile([128, 512], F32, tag="po")
                        for kt in range(NHT):
                            nc.tensor.matmul(
                                pso[:, :],
                                wo_sb[:, kt, bass.ts(dot, 128)],
                                ctxn[:, kt, q0 + j * 512 : q0 + (j + 1) * 512],
                                start=(kt == 0),
                                stop=(kt == NHT - 1),
                            )
                        osb = outp.tile([128, 512], F32, tag="osb")
                        nc.vector.tensor_copy(out=osb, in_=pso)
                        nc.sync.dma_start(
                            out=ot_v[dot, :, q0 + j * 512 : q0 + (j + 1) * 512],
                            in_=osb,
                        )

            # Emission order = dependency order for the deferred (normal
            # priority) projection work. Attention(qp0) is emitted at
            # scheduler priority 0 and preempts as soon as inputs land.
            # K-projection s-blocks all come early: the exp stream walks
            # sk=0..15 and must never starve; V lags slightly (PV is
            # decoupled from exp by the deep pexp pool).
            emit_proj("k", 0)
            for dt in range(NDT):
                nc.sync.dma_start(out=wq_sb[:, dt, :], in_=wq_v[:, dt, :])
            emit_proj("q", 0)
            emit_proj("q", 1)
            for dt in range(NDT):
                nc.sync.dma_start(out=wv_sb[:, dt, :], in_=wv_v[:, dt, :])
            emit_proj("v", 0)
            emit_proj("k", 1)
            emit_proj("v", 1)
            emit_proj("k", 2)
            emit_proj("v", 2)
            emit_proj("k", 3)
            emit_proj("v", 3)
            for kt in range(NHT):
                nc.sync.dma_start(out=wo_sb[:, kt, :], in_=wo_v[:, kt, :])
            emit_proj("q", 2)
            emit_proj("q", 3)
            with tc.high_priority():
                emit_attention(0)
            emit_attention(1)

    nc.compile()
    return nc


_NC_CACHE = None


def _get_nc():
    global _NC_CACHE
    if _NC_CACHE is None:
        _NC_CACHE = build_nc()
    return _NC_CACHE


def make_in_maps(q, k, v, Wq, bq, Wk, bk, Wv, bv, Wo):
    bf = np.float16
    in_maps = []
    for core in range(N_CORES):
        b, hg = core // 2, core % 2
        csl = slice(hg * DHT, (hg + 1) * DHT)
        in_maps.append(
            {
                "xq_t": np.ascontiguousarray(q[b].T).astype(bf),
                "xk_t": np.ascontiguousarray(k[b].T).astype(bf),
                "xv_t": np.ascontiguousarray(v[b].T).astype(bf),
                "wq": np.ascontiguousarray(Wq[:, csl]).astype(bf),
                "wk": np.ascontiguousarray(Wk[:, csl]).astype(bf),
                "wv": np.ascontiguousarray(Wv[:, csl]).astype(bf),
                "wo": np.ascontiguousarray(Wo[csl, :]).astype(bf),
                "bq": np.ascontiguousarray(bq[csl]).astype(np.float32),
                "bk": np.ascontiguousarray(bk[csl]).astype(np.float32),
                "bv": np.ascontiguousarray(bv[csl]).astype(np.float32),
            }
        )
    return in_maps


def kernel(q, k, v, Wq, bq, Wk, bk, Wv, bv, Wo, bo):
    q = np.asarray(q, np.float32)
    k = np.asarray(k, np.float32)
    v = np.asarray(v, np.float32)
    Wq = np.asarray(Wq, np.float32)
    Wk = np.asarray(Wk, np.float32)
    Wv = np.asarray(Wv, np.float32)
    Wo = np.asarray(Wo, np.float32)
    bq = np.asarray(bq, np.float32)
    bk = np.asarray(bk, np.float32)
    bv = np.asarray(bv, np.float32)
    bo = np.asarray(bo, np.float32)

    nc = _get_nc()
    in_maps = make_in_maps(q, k, v, Wq, bq, Wk, bk, Wv, bv, Wo)
    res = run_bass_kernel_spmd(nc, in_maps, list(range(N_CORES)))
    out = np.empty((B, S, D), np.float32)
    for b in range(B):
        o_t = res.results[2 * b]["o_t"] + res.results[2 * b + 1]["o_t"]
        out[b] = o_t.T + bo
    return out


# revision 16
# speedup vs baseline: 1.0377x; 1.0377x over previous
"""Multi-head attention (B=4, S=2048, D=1024, H=16) on 8 trn2 NeuronCores.

Sharding: (batch, head-group) -> 8 shards of (1 batch x 8 heads). Zero
cross-core communication: each core computes Q/K/V projections for its 8
heads, full attention over S=2048, and a partial output projection
(row-split Wo); the host sums the two head-group partials per batch.

v2: head-PAIR packed attention. The PE array is only half-used by a
single head's matmuls (scores contract over dh=64 -> 64 rows; PV emits
65 output cols). Heads 2t/2t+1 are processed together:
  scores: row-tiled pair  -- head A weights in PE rows 0-63, head B in
          rows 64-127 (tile_position auto from base partition); the two
          matmuls stream concurrently.
  PV:     col-tiled pair  -- V_A in PE cols 0-63 -> ctx partitions 0-63,
          V_B in cols 64-127 -> partitions 64-127; one [128,1024] PSUM
          tile holds both heads' ctx.
The softmax denominator (previously a free 65th "ones" column on V, for
which the packed array has no room) is instead computed by a DVE running
sum of the pexp tiles (fp16, 2 elem/cyc) folded by a tiny [128,1]-ones
matmul, then reciprocal + broadcast as before.

Matmul operands are fp16 (fp32 PSUM accumulation; fp16's 10 mantissa
bits survive the PE's internal FP22). fp8 was considered and rejected:
random-walk signals keep the full per-element quantization noise
(~4-6% rms), far over the 2e-2 gate.
"""

import numpy as np

import concourse.bass as bass
import concourse.tile as tile
from concourse import bacc, mybir
from concourse.bass_utils import run_bass_kernel_spmd

F32 = mybir.dt.float32
F16 = mybir.dt.float16
AF = mybir.ActivationFunctionType

B, S, D = 4, 2048, 1024
HPC = 8          # heads per core
DHT = 512        # head dims per core (8 * 64)
NDT = D // 128   # 8 d-tiles (contraction tiles for projections)
NHT = DHT // 128  # 4 head-pairs per core
NST = S // 128   # 16 s-tiles
NSB = S // 512   # 4 s-blocks
N_CORES = 8


def build_nc():
    nc = bacc.Bacc(None, target_bir_lowering=False)

    xq = nc.declare_dram_parameter("xq_t", [D, S], F16, isOutput=False)
    xk = nc.declare_dram_parameter("xk_t", [D, S], F16, isOutput=False)
    xv = nc.declare_dram_parameter("xv_t", [D, S], F16, isOutput=False)
    wq = nc.declare_dram_parameter("wq", [D, DHT], F16, isOutput=False)
    wk = nc.declare_dram_parameter("wk", [D, DHT], F16, isOutput=False)
    wv = nc.declare_dram_parameter("wv", [D, DHT], F16, isOutput=False)
    wo = nc.declare_dram_parameter("wo", [DHT, D], F16, isOutput=False)
    bq = nc.declare_dram_parameter("bq", [DHT], F32, isOutput=False)
    bk = nc.declare_dram_parameter("bk", [DHT], F32, isOutput=False)
    bv = nc.declare_dram_parameter("bv", [DHT], F32, isOutput=False)
    ot = nc.declare_dram_parameter("o_t", [D, S], F32, isOutput=True)

    # DRAM views tiled to 128 partitions
    xq_v = xq.rearrange("(t p) s -> p t s", p=128)
    xk_v = xk.rearrange("(t p) s -> p t s", p=128)
    xv_v = xv.rearrange("(t p) s -> p t s", p=128)
    wq_v = wq.rearrange("(t p) n -> p t n", p=128)
    wk_v = wk.rearrange("(t p) n -> p t n", p=128)
    wv_v = wv.rearrange("(t p) n -> p t n", p=128)
    wo_v = wo.rearrange("(t p) n -> p t n", p=128)
    ot_v = ot.rearrange("(t p) s -> t p s", p=128)

    with tile.TileContext(nc) as tc:
        with (
            tc.tile_pool(name="persist", bufs=1) as persist,
            tc.tile_pool(name="outp", bufs=4) as outp,
            tc.tile_pool(name="w3", bufs=1) as w3,
            tc.tile_pool(name="xs", bufs=4) as xs,
            tc.tile_pool(name="pexp_p", bufs=8) as pexp_p,
            tc.tile_pool(name="accp", bufs=4) as accp,
            tc.tile_pool(name="norm", bufs=1) as norm,
            tc.tile_pool(name="small", bufs=2) as small,
            tc.tile_pool(name="ps_st", bufs=2, space="PSUM") as ps_st,
            tc.tile_pool(name="ps_ctx", bufs=1, space="PSUM") as ps_ctx,
            tc.tile_pool(name="ps_o", bufs=2, space="PSUM") as ps_o,
        ):
            KT = persist.tile([128, NHT, S], F16)        # K^T  [dh-pair, s]
            QT = persist.tile([128, NHT, S], F16)        # Q^T  [dh-pair, s]
            Vt = persist.tile([128, NST, NHT, 128], F16)  # V natural, pair-packed
            ctxn = persist.tile([128, NHT, S], F16)      # normalized ctx^T
            wo_sb = persist.tile([128, NHT, D], F16)
            bq_sb = persist.tile([128, NHT], F32)
            bk_sb = persist.tile([128, NHT], F32)
            bv_bc = persist.tile([128, NHT, 128], F32)
            ones = persist.tile([128, 1], F16)           # den fold column
            denrow = persist.tile([1, 2048], F32)
            rinvrow = persist.tile([1, 2048], F32)
            wq_sb = w3.tile([128, NDT, DHT], F16)
            wk_sb = w3.tile([128, NDT, DHT], F16)
            wv_sb = w3.tile([128, NDT, DHT], F16)

            nc.sync.dma_start(out=bq_sb, in_=bq.rearrange("(t p) -> p t", p=128))
            nc.sync.dma_start(out=bk_sb, in_=bk.rearrange("(t p) -> p t", p=128))
            nc.sync.dma_start(
                out=bv_bc,
                in_=bv.rearrange("(hp c) -> hp c", c=128).partition_broadcast(128),
            )
            nc.vector.memset(ones, 1.0)
            for dt in range(NDT):  # split DMAs across queues; K first
                nc.sync.dma_start(out=wk_sb[:, dt, :], in_=wk_v[:, dt, :])

            def emit_proj(kind, sb):
                ssl = slice(sb * 512, (sb + 1) * 512)
                xv_ap = {"k": xk_v, "v": xv_v, "q": xq_v}[kind]
                xst = xs.tile([128, NDT, 512], F16, tag="xs")
                for dt in range(NDT):
                    nc.sync.dma_start(out=xst[:, dt, :], in_=xv_ap[:, dt, ssl])
                if kind == "v":
                    # V projection, natural layout: the X^T tile is
                    # stationary so out[s-tile, dh] has s on partitions
                    for su in range(4):
                        pso = ps_o.tile([128, 512], F32, tag="po")
                        for dt in range(NDT):
                            nc.tensor.matmul(
                                pso[:, :],
                                xst[:, dt, bass.ts(su, 128)],
                                wv_sb[:, dt, :],
                                start=(dt == 0),
                                stop=(dt == NDT - 1),
                            )
                        nc.vector.tensor_add(
                            out=Vt[:, sb * 4 + su, :, :],
                            in0=pso.rearrange("p (hp c) -> p hp c", c=128),
                            in1=bv_bc,
                        )
                else:
                    w_sb = wk_sb if kind == "k" else wq_sb
                    dst = KT if kind == "k" else QT
                    b_sb = bk_sb if kind == "k" else bq_sb
                    # K^T / Q^T: out[dh-tile, s-blk] = W^T-contract X^T
                    for ht in range(NHT):
                        ps = ps_o.tile([128, 512], F32, tag="po")
                        for dt in range(NDT):
                            nc.tensor.matmul(
                                ps[:, :],
                                w_sb[:, dt, bass.ts(ht, 128)],
                                xst[:, dt, :],
                                start=(dt == 0),
                                stop=(dt == NDT - 1),
                            )
                        # DVE (not ACT) so the scalar engine stays
                        # free for the softmax exps
                        nc.vector.tensor_scalar_add(
                            out=dst[:, ht, ssl],
                            in0=ps[:, :],
                            scalar1=b_sb[:, ht : ht + 1],
                        )

            def emit_attention(qp):
                q0 = qp * 1024
                qsl = slice(q0, q0 + 1024)
                for hp in range(NHT):
                    ctx = ps_ctx.tile([128, 1024], F32, tag="ctx")
                    accA = accp.tile([128, 1024], F16, tag="acc")
                    accB = accp.tile([128, 1024], F16, tag="acc")
                    for sk in range(NST):
                        stA = ps_st.tile([128, 1024], F32, tag="st")
                        stB = ps_st.tile([128, 1024], F32, tag="st")
                        ksl = bass.ts(sk, 128)
                        # Row-tiled pairs (head A rows 0-63, B rows 64-127).
                        # j0/j1 form one accumulation group per st tile so
                        # only the final MM carries a completion signal, and
                        # B (whose buffer-recycle wait is the strictest) goes
                        # first so A's waits dedupe away -- both required for
                        # the pair to stream concurrently in the PE array.
                        for j in range(2):
                            jq = slice(q0 + j * 512, q0 + (j + 1) * 512)
                            nc.tensor.matmul(
                                stB[:, bass.ts(j, 512)],
                                KT[64:128, hp, ksl],
                                QT[64:128, hp, jq],
                                start=True,
                                stop=(j == 1),
                                skip_group_check=True,
                            )
                            nc.tensor.matmul(
                                stA[:, bass.ts(j, 512)],
                                KT[0:64, hp, ksl],
                                QT[0:64, hp, jq],
                                start=True,
                                stop=(j == 1),
                                skip_group_check=True,
                            )
                        pexpA = pexp_p.tile([128, 1024], F16, tag="pexp")
                        pexpB = pexp_p.tile([128, 1024], F16, tag="pexp")
                        nc.scalar.activation(
                            out=pexpA, in_=stA, func=AF.Exp, scale=0.125
                        )
                        nc.scalar.activation(
                            out=pexpB, in_=stB, func=AF.Exp, scale=0.125
                        )
                        # softmax-denominator partial sums (free-dim cheap
                        # on DVE fp16; folded across partitions at the end)
                        # den partial sums (DVE, fp16 2 elem/cyc)
                        if sk == 0:
                            nc.vector.tensor_copy(out=accA, in_=pexpA)
                            nc.vector.tensor_copy(out=accB, in_=pexpB)
                        else:
                            nc.vector.tensor_add(out=accA, in0=accA, in1=pexpA)
                            nc.vector.tensor_add(out=accB, in0=accB, in1=pexpB)
                        for j in range(2):
                            # col-tiled pair: ctx[0:64]=head A, [64:128]=head B
                            nc.tensor.matmul(
                                ctx[0:64, bass.ts(j, 512)],
                                Vt[:, sk, hp, 0:64],
                                pexpA[:, bass.ts(j, 512)],
                                start=(sk == 0),
                                stop=(sk == NST - 1),
                                skip_group_check=True,
                            )
                            nc.tensor.matmul(
                                ctx[64:128, bass.ts(j, 512)],
                                Vt[:, sk, hp, 64:128],
                                pexpB[:, bass.ts(j, 512)],
                                start=(sk == 0),
                                stop=(sk == NST - 1),
                                skip_group_check=True,
                            )
                    # drain ctx PSUM quickly to free the bank for the next
                    # pair; fold acc across partitions into den rows with a
                    # [128,1]-ones matmul; reciprocal runs 128-wide after a
                    # DMA reshape, then per-head halves broadcast back.
                    ctxcA = norm.tile([64, 1024], F32, tag="ctxcA")
                    ctxcB = norm.tile([64, 1024], F32, tag="ctxcB")
                    nc.vector.tensor_copy(out=ctxcA, in_=ctx[0:64, :])
                    nc.vector.tensor_copy(out=ctxcB, in_=ctx[64:128, :])
                    for hd, acc in ((0, accA), (1, accB)):
                        for j in range(2):
                            psd = ps_o.tile([1, 512], F32, tag="po")
                            nc.tensor.matmul(
                                psd[:, :],
                                ones,
                                acc[:, bass.ts(j, 512)],
                                start=True,
                                stop=True,
                            )
                            nc.vector.tensor_copy(
                                out=denrow[:, hd * 1024 + j * 512 : hd * 1024 + (j + 1) * 512],
                                in_=psd,
                            )
                    rr = small.tile([128, 16], F32, tag="rr")
                    nc.sync.dma_start(out=rr, in_=denrow)
                    rrv = small.tile([128, 16], F32, tag="rrv")
                    nc.vector.reciprocal(out=rrv, in_=rr)
                    nc.sync.dma_start(out=rinvrow, in_=rrv)
                    rbcA = norm.tile([64, 1024], F32, tag="rbcA")
                    rbcB = norm.tile([64, 1024], F32, tag="rbcB")
                    nc.gpsimd.partition_broadcast(rbcA, rinvrow[:, 0:1024])
                    nc.gpsimd.partition_broadcast(rbcB, rinvrow[:, 1024:2048])
                    nc.vector.tensor_mul(
                        out=ctxn[0:64, hp, qsl], in0=ctxcA, in1=rbcA
                    )
                    nc.vector.tensor_mul(
                        out=ctxn[64:128, hp, qsl], in0=ctxcB, in1=rbcB
                    )
                # output projection for this q-block
                for dot in range(8):
                    for j in range(2):
                        pso = ps_o.tile([128, 512], F32, tag="po")
                        for kt in range(NHT):
                            nc.tensor.matmul(
                                pso[:, :],
                                wo_sb[:, kt, bass.ts(dot, 128)],
                                ctxn[:, kt, q0 + j * 512 : q0 + (j + 1) * 512],
                                start=(kt == 0),
                                stop=(kt == NHT - 1),
                            )
                        osb = outp.tile([128, 512], F32, tag="osb")
                        nc.vector.tensor_copy(out=osb, in_=pso)
                        nc.sync.dma_start(
                            out=ot_v[dot, :, q0 + j * 512 : q0 + (j + 1) * 512],
                            in_=osb,
                        )

            # Emission order = dependency order for the deferred (normal
            # priority) projection work. Attention(qp0) is emitted at
            # scheduler priority 0 and preempts as soon as inputs land.
            # K-projection s-blocks all come early: the exp stream walks
            # sk=0..15 and must never starve; V lags slightly (PV is
            # decoupled from exp by the deep pexp pool).
            emit_proj("k", 0)
            for dt in range(NDT):
                nc.sync.dma_start(out=wq_sb[:, dt, :], in_=wq_v[:, dt, :])
            emit_proj("q", 0)
            emit_proj("q", 1)
            for dt in range(NDT):
                nc.sync.dma_start(out=wv_sb[:, dt, :], in_=wv_v[:, dt, :])
            emit_proj("v", 0)
            emit_proj("k", 1)
            emit_proj("v", 1)
            emit_proj("k", 2)
            emit_proj("v", 2)
            emit_proj("k", 3)
            emit_proj("v", 3)
            for kt in range(NHT):
                nc.sync.dma_start(out=wo_sb[:, kt, :], in_=wo_v[:, kt, :])
            emit_proj("q", 2)
            emit_proj("q", 3)
            with tc.high_priority():
                emit_attention(0)
            emit_attention(1)

    nc.compile()
    return nc


_NC_CACHE = None


def _get_nc():
    global _NC_CACHE
    if _NC_CACHE is None:
        _NC_CACHE = build_nc()
    return _NC_CACHE


def make_in_maps(q, k, v, Wq, bq, Wk, bk, Wv, bv, Wo):
    bf = np.float16
    in_maps = []
    for core in range(N_CORES):
        b, hg = core // 2, core % 2
        csl = slice(hg * DHT, (hg + 1) * DHT)
        in_maps.append(
            {
                "xq_t": np.ascontiguousarray(q[b].T).astype(bf),
                "xk_t": np.ascontiguousarray(k[b].T).astype(bf),
                "xv_t": np.ascontiguousarray(v[b].T).astype(bf),
                "wq": np.ascontiguousarray(Wq[:, csl]).astype(bf),
                "wk": np.ascontiguousarray(Wk[:, csl]).astype(bf),
                "wv": np.ascontiguousarray(Wv[:, csl]).astype(bf),
                "wo": np.ascontiguousarray(Wo[csl, :]).astype(bf),
                "bq": np.ascontiguousarray(bq[csl]).astype(np.float32),
                "bk": np.ascontiguousarray(bk[csl]).astype(np.float32),
                "bv": np.ascontiguousarray(bv[csl]).astype(np.float32),
            }
        )
    return in_maps


def kernel(q, k, v, Wq, bq, Wk, bk, Wv, bv, Wo, bo):
    q = np.asarray(q, np.float32)
    k = np.asarray(k, np.float32)
    v = np.asarray(v, np.float32)
    Wq = np.asarray(Wq, np.float32)
    Wk = np.asarray(Wk, np.float32)
    Wv = np.asarray(Wv, np.float32)
    Wo = np.asarray(Wo, np.float32)
    bq = np.asarray(bq, np.float32)
    bk = np.asarray(bk, np.float32)
    bv = np.asarray(bv, np.float32)
    bo = np.asarray(bo, np.float32)

    nc = _get_nc()
    in_maps = make_in_maps(q, k, v, Wq, bq, Wk, bk, Wv, bv, Wo)
    res = run_bass_kernel_spmd(nc, in_maps, list(range(N_CORES)))
    out = np.empty((B, S, D), np.float32)
    for b in range(B):
        o_t = res.results[2 * b]["o_t"] + res.results[2 * b + 1]["o_t"]
        out[b] = o_t.T + bo
    return out


# revision 19
# speedup vs baseline: 1.4537x; 1.4010x over previous
"""Multi-head attention (B=4, S=2048, D=1024, H=16) on 8 trn2 NeuronCores.

Sharding: (batch, head-group) -> 8 shards of (1 batch x 8 heads). Zero
cross-core communication: each core computes Q/K/V projections for its 8
heads, full attention over S=2048, and a partial output projection
(row-split Wo); the host sums the two head-group partials per batch.

v2: head-PAIR packed attention. The PE array is only half-used by a
single head's matmuls (scores contract over dh=64 -> 64 rows; PV emits
65 output cols). Heads 2t/2t+1 are processed together:
  scores: row-tiled pair  -- head A weights in PE rows 0-63, head B in
          rows 64-127 (tile_position auto from base partition); the two
          matmuls stream concurrently.
  PV:     col-tiled pair  -- V_A in PE cols 0-63 -> ctx partitions 0-63,
          V_B in cols 64-127 -> partitions 64-127; one [128,1024] PSUM
          tile holds both heads' ctx.
The softmax denominator (previously a free 65th "ones" column on V, for
which the packed array has no room) is instead computed by a DVE running
sum of the pexp tiles (fp16, 2 elem/cyc) folded by a tiny [128,1]-ones
matmul, then reciprocal + broadcast as before.

Matmul operands are fp16 (fp32 PSUM accumulation; fp16's 10 mantissa
bits survive the PE's internal FP22). fp8 was considered and rejected:
random-walk signals keep the full per-element quantization noise
(~4-6% rms), far over the 2e-2 gate.
"""

import numpy as np

import concourse.bass as bass
import concourse.tile as tile
from concourse import bacc, mybir
from concourse.bass_utils import run_bass_kernel_spmd

F32 = mybir.dt.float32
F16 = mybir.dt.float16
AF = mybir.ActivationFunctionType

B, S, D = 4, 2048, 1024
HPC = 8          # heads per core
DHT = 512        # head dims per core (8 * 64)
NDT = D // 128   # 8 d-tiles (contraction tiles for projections)
NHT = DHT // 128  # 4 head-pairs per core
NST = S // 128   # 16 s-tiles
NSB = S // 512   # 4 s-blocks
N_CORES = 8


def build_nc():
    nc = bacc.Bacc(None, target_bir_lowering=False)

    xq = nc.declare_dram_parameter("xq_t", [D, S], F16, isOutput=False)
    xk = nc.declare_dram_parameter("xk_t", [D, S], F16, isOutput=False)
    xv = nc.declare_dram_parameter("xv_t", [D, S], F16, isOutput=False)
    wq = nc.declare_dram_parameter("wq", [D, DHT], F16, isOutput=False)
    wk = nc.declare_dram_parameter("wk", [D, DHT], F16, isOutput=False)
    wv = nc.declare_dram_parameter("wv", [D, DHT], F16, isOutput=False)
    wo = nc.declare_dram_parameter("wo", [DHT, D], F16, isOutput=False)
    bq = nc.declare_dram_parameter("bq", [DHT], F32, isOutput=False)
    bk = nc.declare_dram_parameter("bk", [DHT], F32, isOutput=False)
    bv = nc.declare_dram_parameter("bv", [DHT], F32, isOutput=False)
    ot = nc.declare_dram_parameter("o_t", [D, S], F32, isOutput=True)

    # DRAM views tiled to 128 partitions
    xq_v = xq.rearrange("(t p) s -> p t s", p=128)
    xk_v = xk.rearrange("(t p) s -> p t s", p=128)
    xv_v = xv.rearrange("(t p) s -> p t s", p=128)
    wq_v = wq.rearrange("(t p) n -> p t n", p=128)
    wk_v = wk.rearrange("(t p) n -> p t n", p=128)
    wv_v = wv.rearrange("(t p) n -> p t n", p=128)
    wo_v = wo.rearrange("(t p) n -> p t n", p=128)
    ot_v = ot.rearrange("(t p) s -> t p s", p=128)

    with tile.TileContext(nc) as tc:
        with (
            tc.tile_pool(name="persist", bufs=1) as persist,
            tc.tile_pool(name="outp", bufs=4) as outp,
            tc.tile_pool(name="w3", bufs=1) as w3,
            tc.tile_pool(name="xs", bufs=4) as xs,
            tc.tile_pool(name="pexp_p", bufs=12) as pexp_p,
            tc.tile_pool(name="accp", bufs=4) as accp,
            tc.tile_pool(name="norm", bufs=1) as norm,
            tc.tile_pool(name="small", bufs=2) as small,
            tc.tile_pool(name="ps_st", bufs=2, space="PSUM") as ps_st,
            tc.tile_pool(name="ps_ctx", bufs=1, space="PSUM") as ps_ctx,
            tc.tile_pool(name="ps_o", bufs=2, space="PSUM") as ps_o,
        ):
            KT = persist.tile([128, NHT, S], F16)        # K^T  [dh-pair, s]
            QT = persist.tile([128, NHT, S], F16)        # Q^T  [dh-pair, s]
            Vt = persist.tile([128, NST, NHT, 128], F16)  # V natural, pair-packed
            ctxn = persist.tile([128, NHT, S], F16)      # normalized ctx^T
            wo_sb = persist.tile([128, NHT, D], F16)
            bq_sb = persist.tile([128, NHT], F32)
            bk_sb = persist.tile([128, NHT], F32)
            bv_bc = persist.tile([128, NHT, 128], F32)
            ones = persist.tile([128, 1], F16)           # den fold column
            denrow = persist.tile([1, 2048], F32)
            rinvrow = persist.tile([1, 2048], F32)
            wq_sb = w3.tile([128, NDT, DHT], F16)
            wk_sb = w3.tile([128, NDT, DHT], F16)
            wv_sb = w3.tile([128, NDT, DHT], F16)

            nc.sync.dma_start(out=bq_sb, in_=bq.rearrange("(t p) -> p t", p=128))
            nc.sync.dma_start(out=bk_sb, in_=bk.rearrange("(t p) -> p t", p=128))
            nc.sync.dma_start(
                out=bv_bc,
                in_=bv.rearrange("(hp c) -> hp c", c=128).partition_broadcast(128),
            )
            nc.vector.memset(ones, 1.0)
            for dt in range(NDT):  # split DMAs across queues; K first
                nc.sync.dma_start(out=wk_sb[:, dt, :], in_=wk_v[:, dt, :])

            def emit_proj(kind, sb):
                ssl = slice(sb * 512, (sb + 1) * 512)
                xv_ap = {"k": xk_v, "v": xv_v, "q": xq_v}[kind]
                xst = xs.tile([128, NDT, 512], F16, tag="xs")
                for dt in range(NDT):
                    nc.sync.dma_start(out=xst[:, dt, :], in_=xv_ap[:, dt, ssl])
                if kind == "v":
                    # V projection, natural layout: the X^T tile is
                    # stationary so out[s-tile, dh] has s on partitions
                    for su in range(4):
                        pso = ps_o.tile([128, 512], F32, tag="po")
                        for dt in range(NDT):
                            nc.tensor.matmul(
                                pso[:, :],
                                xst[:, dt, bass.ts(su, 128)],
                                wv_sb[:, dt, :],
                                start=(dt == 0),
                                stop=(dt == NDT - 1),
                            )
                        nc.vector.tensor_add(
                            out=Vt[:, sb * 4 + su, :, :],
                            in0=pso.rearrange("p (hp c) -> p hp c", c=128),
                            in1=bv_bc,
                        )
                else:
                    w_sb = wk_sb if kind == "k" else wq_sb
                    dst = KT if kind == "k" else QT
                    b_sb = bk_sb if kind == "k" else bq_sb
                    # K^T / Q^T: out[dh-tile, s-blk] = W^T-contract X^T
                    for ht in range(NHT):
                        ps = ps_o.tile([128, 512], F32, tag="po")
                        for dt in range(NDT):
                            nc.tensor.matmul(
                                ps[:, :],
                                w_sb[:, dt, bass.ts(ht, 128)],
                                xst[:, dt, :],
                                start=(dt == 0),
                                stop=(dt == NDT - 1),
                            )
                        # DVE (not ACT) so the scalar engine stays
                        # free for the softmax exps
                        nc.vector.tensor_scalar_add(
                            out=dst[:, ht, ssl],
                            in0=ps[:, :],
                            scalar1=b_sb[:, ht : ht + 1],
                        )

            def attn_core(qp, hp, ctx, accA, accB):
                q0 = qp * 1024
                if True:
                    for sk in range(NST):
                        stA = ps_st.tile([128, 1024], F32, tag="st")
                        stB = ps_st.tile([128, 1024], F32, tag="st")
                        ksl = bass.ts(sk, 128)
                        # Head A's two q-halves form one signal group (one
                        # LDW, back-to-back streams, single completion inc)
                        # so expA starts while head B's group still streams
                        # on PE rows 64-127.
                        for po, st_t in ((0, stA), (64, stB)):
                            for j in range(2):
                                jq = slice(q0 + j * 512, q0 + (j + 1) * 512)
                                nc.tensor.matmul(
                                    st_t[:, bass.ts(j, 512)],
                                    KT[po : po + 64, hp, ksl],
                                    QT[po : po + 64, hp, jq],
                                    start=True,
                                    stop=(j == 1),
                                    skip_group_check=True,
                                )
                        pexpA = pexp_p.tile([128, 1024], F16, tag="pexp")
                        pexpB = pexp_p.tile([128, 1024], F16, tag="pexp")
                        nc.scalar.activation(
                            out=pexpA, in_=stA, func=AF.Exp, scale=0.125
                        )
                        nc.scalar.activation(
                            out=pexpB, in_=stB, func=AF.Exp, scale=0.125
                        )
                        # softmax-denominator partial sums (free-dim cheap
                        # on DVE fp16; folded across partitions at the end)
                        # den partial sums (DVE, fp16 2 elem/cyc)
                        if sk == 0:
                            nc.vector.tensor_copy(out=accA, in_=pexpA)
                            nc.vector.tensor_copy(out=accB, in_=pexpB)
                        else:
                            nc.vector.tensor_add(out=accA, in0=accA, in1=pexpA)
                            nc.vector.tensor_add(out=accB, in0=accB, in1=pexpB)
                        for j in range(2):
                            # col-tiled pair: ctx[0:64]=head A, [64:128]=head B
                            nc.tensor.matmul(
                                ctx[0:64, bass.ts(j, 512)],
                                Vt[:, sk, hp, 0:64],
                                pexpA[:, bass.ts(j, 512)],
                                start=(sk == 0),
                                stop=(sk == NST - 1),
                                skip_group_check=True,
                            )
                            nc.tensor.matmul(
                                ctx[64:128, bass.ts(j, 512)],
                                Vt[:, sk, hp, 64:128],
                                pexpB[:, bass.ts(j, 512)],
                                start=(sk == 0),
                                stop=(sk == NST - 1),
                                skip_group_check=True,
                            )
            def attn_finish(qp, hp, ctx, accA, accB):
                q0 = qp * 1024
                qsl = slice(q0, q0 + 1024)
                if True:
                    # drain ctx PSUM quickly to free the bank for the next
                    # pair; fold acc across partitions into den rows with a
                    # [128,1]-ones matmul; reciprocal runs 128-wide after a
                    # DMA reshape, then per-head halves broadcast back.
                    ctxcA = norm.tile([64, 1024], F32, tag="ctxcA")
                    ctxcB = norm.tile([64, 1024], F32, tag="ctxcB")
                    nc.vector.tensor_copy(out=ctxcA, in_=ctx[0:64, :])
                    nc.vector.tensor_copy(out=ctxcB, in_=ctx[64:128, :])
                    for hd, acc in ((0, accA), (1, accB)):
                        for j in range(2):
                            psd = ps_o.tile([1, 512], F32, tag="po")
                            nc.tensor.matmul(
                                psd[:, :],
                                ones,
                                acc[:, bass.ts(j, 512)],
                                start=True,
                                stop=True,
                            )
                            nc.vector.tensor_copy(
                                out=denrow[:, hd * 1024 + j * 512 : hd * 1024 + (j + 1) * 512],
                                in_=psd,
                            )
                    rr = small.tile([128, 16], F32, tag="rr")
                    nc.sync.dma_start(out=rr, in_=denrow)
                    rrv = small.tile([128, 16], F32, tag="rrv")
                    nc.vector.reciprocal(out=rrv, in_=rr)
                    nc.sync.dma_start(out=rinvrow, in_=rrv)
                    rbcA = norm.tile([64, 1024], F32, tag="rbcA")
                    rbcB = norm.tile([64, 1024], F32, tag="rbcB")
                    nc.gpsimd.partition_broadcast(rbcA, rinvrow[:, 0:1024])
                    nc.gpsimd.partition_broadcast(rbcB, rinvrow[:, 1024:2048])
                    nc.vector.tensor_mul(
                        out=ctxn[0:64, hp, qsl], in0=ctxcA, in1=rbcA
                    )
                    nc.vector.tensor_mul(
                        out=ctxn[64:128, hp, qsl], in0=ctxcB, in1=rbcB
                    )

            def attn_oproj(qp):
                q0 = qp * 1024
                # output projection for this q-block
                for dot in range(8):
                    for j in range(2):
                        pso = ps_o.tile([128, 512], F32, tag="po")
                        for kt in range(NHT):
                            nc.tensor.matmul(
                                pso[:, :],
                                wo_sb[:, kt, bass.ts(dot, 128)],
                                ctxn[:, kt, q0 + j * 512 : q0 + (j + 1) * 512],
                                start=(kt == 0),
                                stop=(kt == NHT - 1),
                            )
                        osb = outp.tile([128, 512], F32, tag="osb")
                        nc.vector.tensor_copy(out=osb, in_=pso)
                        nc.sync.dma_start(
                            out=ot_v[dot, :, q0 + j * 512 : q0 + (j + 1) * 512],
                            in_=osb,
                        )

            # Emission order = dependency order for the deferred (normal
            # priority) projection work. Attention(qp0) is emitted at
            # scheduler priority 0 and preempts as soon as inputs land.
            # K-projection s-blocks all come early: the exp stream walks
            # sk=0..15 and must never starve; V lags slightly (PV is
            # decoupled from exp by the deep pexp pool).
            emit_proj("k", 0)
            for dt in range(NDT):
                nc.sync.dma_start(out=wq_sb[:, dt, :], in_=wq_v[:, dt, :])
            emit_proj("q", 0)
            emit_proj("q", 1)
            for dt in range(NDT):
                nc.sync.dma_start(out=wv_sb[:, dt, :], in_=wv_v[:, dt, :])
            emit_proj("v", 0)
            emit_proj("k", 1)
            emit_proj("v", 1)
            emit_proj("k", 2)
            emit_proj("v", 2)
            emit_proj("k", 3)
            emit_proj("v", 3)
            for kt in range(NHT):
                nc.sync.dma_start(out=wo_sb[:, kt, :], in_=wo_v[:, kt, :])
            emit_proj("q", 2)
            emit_proj("q", 3)
            for qp in range(2):
                for hp in range(NHT):
                    ctx = ps_ctx.tile([128, 1024], F32, tag="ctx")
                    accA = accp.tile([128, 1024], F16, tag="acc")
                    accB = accp.tile([128, 1024], F16, tag="acc")
                    with tc.high_priority():
                        attn_core(qp, hp, ctx, accA, accB)
                    attn_finish(qp, hp, ctx, accA, accB)
                attn_oproj(qp)

    nc.compile()
    return nc


_NC_CACHE = None


def _get_nc():
    global _NC_CACHE
    if _NC_CACHE is None:
        _NC_CACHE = build_nc()
    return _NC_CACHE


def make_in_maps(q, k, v, Wq, bq, Wk, bk, Wv, bv, Wo):
    bf = np.float16
    in_maps = []
    for core in range(N_CORES):
        b, hg = core // 2, core % 2
        csl = slice(hg * DHT, (hg + 1) * DHT)
        in_maps.append(
            {
                "xq_t": np.ascontiguousarray(q[b].T).astype(bf),
                "xk_t": np.ascontiguousarray(k[b].T).astype(bf),
                "xv_t": np.ascontiguousarray(v[b].T).astype(bf),
                "wq": np.ascontiguousarray(Wq[:, csl]).astype(bf),
                "wk": np.ascontiguousarray(Wk[:, csl]).astype(bf),
                "wv": np.ascontiguousarray(Wv[:, csl]).astype(bf),
                "wo": np.ascontiguousarray(Wo[csl, :]).astype(bf),
                "bq": np.ascontiguousarray(bq[csl]).astype(np.float32),
                "bk": np.ascontiguousarray(bk[csl]).astype(np.float32),
                "bv": np.ascontiguousarray(bv[csl]).astype(np.float32),
            }
        )
    return in_maps


def kernel(q, k, v, Wq, bq, Wk, bk, Wv, bv, Wo, bo):
    q = np.asarray(q, np.float32)
    k = np.asarray(k, np.float32)
    v = np.asarray(v, np.float32)
    Wq = np.asarray(Wq, np.float32)
    Wk = np.asarray(Wk, np.float32)
    Wv = np.asarray(Wv, np.float32)
    Wo = np.asarray(Wo, np.float32)
    bq = np.asarray(bq, np.float32)
    bk = np.asarray(bk, np.float32)
    bv = np.asarray(bv, np.float32)
    bo = np.asarray(bo, np.float32)

    nc = _get_nc()
    in_maps = make_in_maps(q, k, v, Wq, bq, Wk, bk, Wv, bv, Wo)
    res = run_bass_kernel_spmd(nc, in_maps, list(range(N_CORES)))
    out = np.empty((B, S, D), np.float32)
    for b in range(B):
        o_t = res.results[2 * b]["o_t"] + res.results[2 * b + 1]["o_t"]
        out[b] = o_t.T + bo
    return out
